# revision 22
# baseline (speedup 1.0000x reference)
"""Adaptive Computation Time LSTM cell (B=1024, I=H=2048, 10 steps) on 8
Trainium2 NeuronCores.

Sharding: tensor-parallel over the 4H gate dimension. Core r owns the rows
[g*H + r*256 .. g*H + (r+1)*256) of W_ih/W_hh for each gate g (i,f,g,o), i.e.
a 256-wide slice of the hidden dimension. All state tensors live transposed
([feature, batch]) so the recurrent matmul's contraction dim (H) sits on SBUF
partitions:

  - per step: gates_slice.T [1024, B] = W_cat_slice @ h.T, K = 2048
  - h_new slice [256, B] computed locally (bf16), AllGathered so every core
    holds the full h.T [2048, B] for the next step's matmul
  - the halt probability y = sigmoid(h_new @ W_halt.T + b_halt) comes from
    per-core partial dot products packed into the same AllGather payload; the
    halting state (hp, remainders, n_updates, active mask) is replicated on
    every core, so the global min-halting early-exit check is local.

Each step is pipelined over two batch halves: while one half's h_new is in
flight (AllGather + scatter back to SBUF), the PE computes the other half's
gates, hiding most of the communication.

Correctness trick: the reference's scalar `active` mask freezes h/c after
halting, but once halted every later update is discarded, so h/c can evolve
unmasked; only the accumulators (hp, rem, nup, hsum) need masking.

Recurrent matmuls run in bf16 (weights + h), accumulating fp32 in PSUM; the
input projection runs as float32r (full-rate fp32 mode of the PE array).
"""

import numpy as np
import ml_dtypes

import concourse.bass as bass
import concourse.bacc as bacc
import concourse.bass_isa as bass_isa
import concourse.tile as tile
from concourse import mybir
from concourse.bass_utils import run_bass_kernel_spmd

F32 = mybir.dt.float32
F32R = mybir.dt.float32r
BF16 = mybir.dt.bfloat16
AF = mybir.ActivationFunctionType
ALU = mybir.AluOpType

NC = 8          # cores
B = 1024        # batch
I = 2048        # input dim
H = 2048        # hidden dim
HS = H // NC    # hidden slice per core (256)
SL = 4 * HS     # gate-row slice per core (1024)
KT = 16         # contraction k-tiles of 128 (over I or H)
MT = 8          # M-tiles of 128 over the 1024-row gate slice
PT = HS // 128  # partition-tiles per hidden slice (2)
STEPS = 10
THRESH = np.float32(1.0 - 0.01)
NH = 2          # batch halves per step (pipelined)
NW = B // NH    # 512
MH = MT // NH   # m-tiles per half (4)
BR = 2 * 128 + 1  # bounce rows per half: 256 h rows + 1 partial row
DEBUG = False


def _build():
    nc = bacc.Bacc("TRN2", target_bir_lowering=False, debug=False, num_devices=NC)

    # --- kernel I/O ---
    whhT_d = nc.dram_tensor("whhT", [H, SL], BF16, kind="ExternalInput")
    wihT_d = nc.dram_tensor("wihT", [I, SL], F32, kind="ExternalInput")
    xT_d = nc.dram_tensor("xT", [I, B], F32, kind="ExternalInput")
    bias_d = nc.dram_tensor("bias", [SL], F32, kind="ExternalInput")
    whalt_d = nc.dram_tensor("whalt", [HS], BF16, kind="ExternalInput")
    bhalt_d = nc.dram_tensor("bhalt", [128, 1], F32, kind="ExternalInput")

    avg_d = nc.dram_tensor("avgT", [HS, B], F32, kind="ExternalOutput")
    hp_d = nc.dram_tensor("hp_out", [B], F32, kind="ExternalOutput")
    nup_d = nc.dram_tensor("nup_out", [B], F32, kind="ExternalOutput")
    if DEBUG:
        dbg = {k: nc.dram_tensor(k, shp, F32, kind="ExternalOutput") for k, shp in [
            ("dbg_xg", [SL, B]), ("dbg_h1", [HS, B]), ("dbg_y1", [B]),
            ("dbg_h10", [HS, B]), ("dbg_hsum", [HS, B]), ("dbg_rem", [B]),
            ("dbg_fac", [B])]}

    groups = [list(range(NC))]

    with tile.TileContext(nc) as tc:
        with (
            tc.tile_pool(name="wsb", bufs=1) as wsb,
            tc.tile_pool(name="big", bufs=1) as big,
            tc.tile_pool(name="xgp", bufs=1) as xgp,
            tc.tile_pool(name="st", bufs=1) as st,
            tc.tile_pool(name="ps", bufs=1, space="PSUM") as ps,
            tc.tile_pool(name="dram", bufs=1, space="DRAM") as dram,
        ):
            # --- resident tiles ---
            whh = [wsb.tile([128, SL], BF16, tag=f"whh{k}", name=f"whh{k}")
                   for k in range(KT)]
            for k in range(KT):
                nc.sync.dma_start(whh[k][:], whhT_d[128 * k:128 * (k + 1), :])

            xg = [xgp.tile([128, B], F32, tag=f"xg{m}", name=f"xg{m}") for m in range(MT)]

            bias_sb = st.tile([128, MT], F32, name="bias_sb")
            nc.sync.dma_start(bias_sb[:], bias_d.ap().rearrange("(m p) -> p m", p=128))
            whalt_sb = st.tile([128, PT], BF16, name="whalt_sb")
            nc.sync.dma_start(whalt_sb[:], whalt_d.ap().rearrange("(k p) -> p k", p=128))
            bhalt_sb = st.tile([128, 1], F32, name="bhalt_sb")
            nc.sync.dma_start(bhalt_sb[:], bhalt_d[:, :])

            c_t = [st.tile([128, B], F32, tag=f"c{p}", name=f"c{p}") for p in range(PT)]
            hs_t = [st.tile([128, B], F32, tag=f"hs{p}", name=f"hs{p}") for p in range(PT)]
            hn_t = [st.tile([128, B], BF16, tag=f"hn{p}", name=f"hn{p}") for p in range(PT)]
            for p in range(PT):
                nc.vector.memset(c_t[p][:], 0.0)
                nc.vector.memset(hs_t[p][:], 0.0)
            hp_t = st.tile([128, 8], F32, name="hp_t")
            rem_t = st.tile([128, 8], F32, name="rem_t")
            nup_t = st.tile([128, 8], F32, name="nup_t")
            m128 = st.tile([128, 1], F32, name="m128")
            nc.vector.memset(hp_t[:], 0.0)
            nc.vector.memset(rem_t[:], 0.0)
            nc.vector.memset(nup_t[:], 0.0)
            nc.vector.memset(m128[:], 1.0)

            # --- phase 1: xg = W_ih_slice @ x.T + bias (f32r, into resident xg) ---
            xt = [big.tile([128, B], F32R, tag=f"h{k}", name=f"xt{k}") for k in range(KT)]
            for k in range(KT):
                nc.sync.dma_start(xt[k][:], xT_d[128 * k:128 * (k + 1), :].bitcast(F32R))
            with tc.tile_pool(name="wih", bufs=1) as wihp:
                for n in range(NH):
                    nsl = slice(n * NW, (n + 1) * NW)
                    pst = [ps.tile([128, NW], F32, tag="ps", bufs=8, name=f"psxg{n}_{m}")
                           for m in range(MT)]
                    for k in range(KT):
                        wt = wihp.tile([128, SL], F32R, tag="wih", bufs=3,
                                       name=f"wih{n}_{k}")
                        nc.sync.dma_start(wt[:],
                                          wihT_d[128 * k:128 * (k + 1), :].bitcast(F32R))
                        for m in range(MT):
                            nc.tensor.matmul(pst[m][:], wt[:, 128 * m:128 * (m + 1)],
                                             xt[k][:, nsl],
                                             start=(k == 0), stop=(k == KT - 1))
                    for m in range(MT):
                        nc.scalar.activation(xg[m][:, nsl], pst[m][:], AF.Identity,
                                             bias=bias_sb[:, m:m + 1])

            # --- phase 2: recurrent steps, pipelined over batch halves ---
            tp_cm = tc.tile_pool(name="tmp", bufs=1)
            tp = tp_cm.__enter__()
            hT = [None] * KT
            yp_all = None
            for t in range(1, STEPS + 1):
                hT_next = [big.tile([128, B], BF16, tag=f"h{k}", name=f"hT{t}_{k}")
                           for k in range(KT)] if t < STEPS else None
                yp_next = st.tile([128, 64], BF16, tag="ypall", bufs=2, name=f"ypall{t}")
                for n in range(NH):
                    nsl = slice(n * NW, (n + 1) * NW)
                    # (A) gate waves for this half
                    for p in range(PT):
                        tmps = []
                        if t == 1:
                            for gi, gname in enumerate("ifgo"):
                                m = 2 * gi + p
                                tt = tp.tile([128, NW], F32, tag=f"t{gi}", bufs=2,
                                             name=f"t{t}_{p}_{n}_{gi}")
                                nc.scalar.activation(
                                    tt[:], xg[m][:, nsl],
                                    AF.Tanh if gname == "g" else AF.Sigmoid)
                                tmps.append(tt)
                        else:
                            pst = [ps.tile([128, NW], F32, tag="ps", bufs=8,
                                           name=f"psg{t}_{p}_{n}_{gi}")
                                   for gi in range(4)]
                            for k in range(KT):
                                for gi in range(4):
                                    m = 2 * gi + p
                                    nc.tensor.matmul(
                                        pst[gi][:],
                                        whh[k][:, 128 * m:128 * (m + 1)],
                                        hT[k][:, nsl],
                                        start=(k == 0), stop=(k == KT - 1))
                            for gi, gname in enumerate("ifgo"):
                                m = 2 * gi + p
                                tt = tp.tile([128, NW], F32, tag=f"t{gi}", bufs=2,
                                             name=f"t{t}_{p}_{n}_{gi}")
                                nc.vector.tensor_add(tt[:], pst[gi][:], xg[m][:, nsl])
                                nc.scalar.activation(
                                    tt[:], tt[:],
                                    AF.Tanh if gname == "g" else AF.Sigmoid)
                                tmps.append(tt)
                        t_i, t_f, t_g, t_o = tmps
                        nc.vector.tensor_mul(t_i[:], t_i[:], t_g[:])
                        nc.vector.tensor_mul(t_f[:], t_f[:], c_t[p][:, nsl])
                        nc.vector.tensor_add(c_t[p][:, nsl], t_f[:], t_i[:])
                        nc.scalar.activation(t_g[:], c_t[p][:, nsl], AF.Tanh)
                        nc.vector.tensor_mul(hn_t[p][:, nsl], t_o[:], t_g[:])
                        nc.vector.scalar_tensor_tensor(
                            hs_t[p][:, nsl], hn_t[p][:, nsl], m128[:, :],
                            hs_t[p][:, nsl], op0=ALU.mult, op1=ALU.add)

                    # (B) halt partial logits for this half's batch m-tiles
                    yps = ps.tile([128, MH], F32, tag="ps", bufs=8, name=f"yps{t}_{n}")
                    for mi in range(MH):
                        m = n * MH + mi
                        for p in range(PT):
                            nc.tensor.matmul(yps[:, mi:mi + 1],
                                             hn_t[p][:, 128 * m:128 * (m + 1)],
                                             whalt_sb[:, p:p + 1],
                                             start=(p == 0), stop=(p == PT - 1))
                    part_bf = st.tile([128, MH], BF16, tag="part", bufs=2,
                                      name=f"part{t}_{n}")
                    nc.vector.tensor_copy(part_bf[:], yps[:])

                    # (C) exchange: h_new half (t < STEPS) + halt partials
                    if t < STEPS:
                        bounce = dram.tile([BR, NW], BF16, tag="bounce", bufs=2,
                                           name=f"bounce{t}_{n}")
                        for p in range(PT):
                            nc.sync.dma_start(bounce[128 * p:128 * (p + 1), :],
                                              hn_t[p][:, nsl])
                        nc.sync.dma_start(
                            bounce[2 * 128:BR, :].rearrange("o (m q) -> (o q) m", q=128),
                            part_bf[:])
                        agout = dram.tile([NC * BR, NW], BF16, tag="agout", bufs=2,
                                          addr_space="Shared", name=f"agout{t}_{n}")
                        nc.gpsimd.collective_compute(
                            "AllGather", ALU.bypass, replica_groups=groups,
                            ins=[bounce.opt()], outs=[agout.opt()])
                        for k in range(KT):
                            row = (k // PT) * BR + (k % PT) * 128
                            nc.sync.dma_start(hT_next[k][:, nsl],
                                              agout[row:row + 128, :])
                        for r in range(NC):
                            row = r * BR + 2 * 128
                            nc.sync.dma_start(
                                yp_next[:, 8 * r + MH * n:8 * r + MH * (n + 1)],
                                agout[row:row + 1, :].rearrange(
                                    "o (m q) -> (o q) m", q=128))
                    else:
                        bounce = dram.tile([1, NW], BF16, tag="bouncey", bufs=2,
                                           name=f"bouncey{n}")
                        nc.sync.dma_start(
                            bounce[:, :].rearrange("o (m q) -> (o q) m", q=128),
                            part_bf[:])
                        agout = dram.tile([NC, NW], BF16, tag="agouty", bufs=2,
                                          addr_space="Shared", name=f"agouty{n}")
                        nc.gpsimd.collective_compute(
                            "AllGather", ALU.bypass, replica_groups=groups,
                            ins=[bounce.opt()], outs=[agout.opt()])
                        for r in range(NC):
                            nc.sync.dma_start(
                                yp_next[:, 8 * r + MH * n:8 * r + MH * (n + 1)],
                                agout[r:r + 1, :].rearrange("o (m q) -> (o q) m", q=128))

                # (D) per-step halting-state update (both halves' partials in)
                logit = st.tile([128, 8], F32, tag="logit", name=f"logit{t}")
                nc.vector.tensor_reduce(
                    logit[:], yp_next[:, :].rearrange("p (r j) -> p j r", r=NC),
                    axis=mybir.AxisListType.X, op=ALU.add)
                y_t = st.tile([128, 8], F32, tag="y_t", name=f"y{t}")
                nc.scalar.activation(y_t[:], logit[:], AF.Sigmoid, bias=bhalt_sb[:, :])
                t1 = st.tile([128, 8], F32, tag="t1", name=f"t1_{t}")
                nc.vector.tensor_scalar(t1[:], hp_t[:], -1.0, 1.0, op0=ALU.mult, op1=ALU.add)
                nc.vector.tensor_mul(t1[:], y_t[:], t1[:])          # y*(1-hp)
                nc.vector.scalar_tensor_tensor(hp_t[:], t1[:], m128[:, :], hp_t[:],
                                               op0=ALU.mult, op1=ALU.add)
                nc.vector.tensor_scalar(t1[:], hp_t[:], -1.0, 1.0, op0=ALU.mult, op1=ALU.add)
                nc.vector.scalar_tensor_tensor(rem_t[:], t1[:], m128[:, :], rem_t[:],
                                               op0=ALU.mult, op1=ALU.add)
                nc.vector.tensor_scalar_add(nup_t[:], nup_t[:], m128[:, :])
                # active &= (min(hp) <= THRESH), via -max(-hp)
                hmin = st.tile([128, 1], F32, tag="hmin", name=f"hmin{t}")
                nc.vector.tensor_reduce(hmin[:], hp_t[:], axis=mybir.AxisListType.X,
                                        op=ALU.min)
                nc.vector.tensor_scalar_mul(hmin[:], hmin[:], -1.0)
                nall = st.tile([128, 1], F32, tag="nall", name=f"nall{t}")
                nc.gpsimd.partition_all_reduce(nall[:], hmin[:], channels=128,
                                               reduce_op=bass_isa.ReduceOp.max)
                nc.vector.tensor_scalar(nall[:], nall[:], float(-THRESH), None,
                                        op0=ALU.is_ge)
                nc.vector.tensor_mul(m128[:], m128[:], nall[:])
                hT = hT_next
                if DEBUG and t == 1:
                    for p in range(PT):
                        nc.gpsimd.dma_start(dbg["dbg_h1"][128 * p:128 * (p + 1), :],
                                            hn_t[p][:])
                    nc.sync.dma_start(dbg["dbg_y1"].ap().rearrange("(m q) -> q m", q=128),
                                      y_t[:])
                if DEBUG and t == STEPS:
                    for p in range(PT):
                        nc.gpsimd.dma_start(dbg["dbg_h10"][128 * p:128 * (p + 1), :],
                                            hn_t[p][:])
                        nc.sync.dma_start(dbg["dbg_hsum"][128 * p:128 * (p + 1), :],
                                          hs_t[p][:])
                    nc.sync.dma_start(dbg["dbg_rem"].ap().rearrange("(m q) -> q m", q=128),
                                      rem_t[:])

            # --- phase 3: outputs ---
            fac = st.tile([128, 8], F32, tag="fac", name="fac")
            nc.vector.reciprocal(fac[:], nup_t[:])
            nc.vector.tensor_mul(fac[:], rem_t[:], fac[:])
            fbuf = dram.tile([B], F32, tag="fbuf", name="fbuf")
            nc.sync.dma_start(fbuf[:].rearrange("(m q) -> q m", q=128), fac[:])
            if DEBUG:
                nc.sync.dma_start(dbg["dbg_fac"].ap().rearrange("(m q) -> q m", q=128),
                                  fac[:])
                for m in range(MT):
                    nc.sync.dma_start(dbg["dbg_xg"][128 * m:128 * (m + 1), :], xg[m][:])
            frow = tp.tile([1, B], F32, tag="frow", name="frow")
            nc.sync.dma_start(frow[:], fbuf[:].rearrange("(o b) -> o b", o=1))
            ones_sb = tp.tile([1, 128], F32, tag="ones", name="ones_sb")
            nc.vector.memset(ones_sb[:], 1.0)
            # broadcast factor over partitions via a K=1 matmul, multiply in place
            for n in range(NH):
                nsl = slice(n * NW, (n + 1) * NW)
                fps = ps.tile([128, NW], F32, tag="ps", bufs=8, name=f"fps{n}")
                nc.tensor.matmul(fps[:], ones_sb[:], frow[:, nsl], start=True, stop=True)
                for p in range(PT):
                    av = xgp.tile([128, NW], F32, tag=f"xg{2 * n + p}", name=f"av{n}_{p}")
                    nc.vector.tensor_mul(av[:], hs_t[p][:, nsl], fps[:])
                    nc.sync.dma_start(avg_d[128 * p:128 * (p + 1), nsl], av[:])
            nc.sync.dma_start(hp_d.ap().rearrange("(m q) -> q m", q=128), hp_t[:])
            nc.sync.dma_start(nup_d.ap().rearrange("(m q) -> q m", q=128), nup_t[:])
            tp_cm.__exit__(None, None, None)

    nc.compile()
    return nc


def _shard(x, W_ih, W_hh, b_ih, b_hh, W_halt, b_halt):
    xT = np.ascontiguousarray(x.T)
    bsum = b_ih + b_hh
    in_maps = []
    for r in range(NC):
        rows = np.concatenate([np.arange(g * H + r * HS, g * H + (r + 1) * HS)
                               for g in range(4)])
        in_maps.append({
            "whhT": np.ascontiguousarray(W_hh[rows].T).astype(ml_dtypes.bfloat16),
            "wihT": np.ascontiguousarray(W_ih[rows].T),
            "xT": xT,
            "bias": np.ascontiguousarray(bsum[rows]),
            "whalt": np.ascontiguousarray(
                W_halt[0, r * HS:(r + 1) * HS]).astype(ml_dtypes.bfloat16),
            "bhalt": np.full((128, 1), np.float32(b_halt[0]), np.float32),
        })
    return in_maps


def _run(inputs, trace=False):
    nc = _build()
    in_maps = _shard(**{k: np.asarray(inputs[k], np.float32) for k in
                        ("x", "W_ih", "W_hh", "b_ih", "b_hh", "W_halt", "b_halt")})
    res = run_bass_kernel_spmd(nc, in_maps, core_ids=list(range(NC)), trace=trace)
    avg = np.empty((B, H), np.float32)
    for r in range(NC):
        avg[:, r * HS:(r + 1) * HS] = res.results[r]["avgT"].T
    hp = res.results[0]["hp_out"].reshape(B, 1).astype(np.float32)
    nup = res.results[0]["nup_out"].reshape(B, 1).astype(np.float32)
    return (avg, hp, nup), res


def kernel(**inputs):
    out, _ = _run(inputs, trace=False)
    return out


# revision 23
# speedup vs baseline: 1.2170x; 1.2170x over previous
"""Adaptive Computation Time LSTM cell (B=1024, I=H=2048, 10 steps) on 8
Trainium2 NeuronCores.

Sharding: tensor-parallel over the 4H gate dimension. Core r owns the rows
[g*H + r*256 .. g*H + (r+1)*256) of W_ih/W_hh for each gate g (i,f,g,o), i.e.
a 256-wide slice of the hidden dimension. All state tensors live transposed
([feature, batch]) so the recurrent matmul's contraction dim (H) sits on SBUF
partitions:

  - per step: gates_slice.T [1024, B] = W_cat_slice @ h.T, K = 2048
  - h_new slice [256, B] computed locally (bf16), AllGathered so every core
    holds the full h.T [2048, B] for the next step's matmul
  - the halt probability y = sigmoid(h_new @ W_halt.T + b_halt) comes from
    per-core partial dot products packed into the same AllGather payload; the
    halting state (hp, remainders, n_updates, active mask) is replicated on
    every core, so the global min-halting early-exit check is local.

Each step is pipelined over two batch halves: while one half's h_new is in
flight (AllGather + scatter back to SBUF), the PE computes the other half's
gates, hiding most of the communication.

Correctness trick: the reference's scalar `active` mask freezes h/c after
halting, but once halted every later update is discarded, so h/c can evolve
unmasked; only the accumulators (hp, rem, nup, hsum) need masking.

Recurrent matmuls run in bf16 (weights + h), accumulating fp32 in PSUM; the
input projection runs as float32r (full-rate fp32 mode of the PE array).
"""

import numpy as np
import ml_dtypes

import concourse.bass as bass
import concourse.bacc as bacc
import concourse.bass_isa as bass_isa
import concourse.tile as tile
from concourse import mybir
from concourse.bass_utils import run_bass_kernel_spmd

F32 = mybir.dt.float32
F32R = mybir.dt.float32r
BF16 = mybir.dt.bfloat16
AF = mybir.ActivationFunctionType
ALU = mybir.AluOpType

NC = 8          # cores
B = 1024        # batch
I = 2048        # input dim
H = 2048        # hidden dim
HS = H // NC    # hidden slice per core (256)
SL = 4 * HS     # gate-row slice per core (1024)
KT = 16         # contraction k-tiles of 128 (over I or H)
MT = 8          # M-tiles of 128 over the 1024-row gate slice
PT = HS // 128  # partition-tiles per hidden slice (2)
STEPS = 10
THRESH = np.float32(1.0 - 0.01)
NH = 2          # batch halves per step (pipelined)
NW = B // NH    # 512
MH = MT // NH   # m-tiles per half (4)
BR = 2 * 128 + 1  # bounce rows per half: 256 h rows + 1 partial row
DEBUG = False


def _build():
    nc = bacc.Bacc("TRN2", target_bir_lowering=False, debug=False, num_devices=NC)

    # --- kernel I/O ---
    whhT_d = nc.dram_tensor("whhT", [H, SL], BF16, kind="ExternalInput")
    wihT_d = nc.dram_tensor("wihT", [I, SL], F32, kind="ExternalInput")
    xT_d = nc.dram_tensor("xT", [I, B], F32, kind="ExternalInput")
    bias_d = nc.dram_tensor("bias", [SL], F32, kind="ExternalInput")
    whalt_d = nc.dram_tensor("whalt", [HS], BF16, kind="ExternalInput")
    bhalt_d = nc.dram_tensor("bhalt", [128, 1], F32, kind="ExternalInput")

    avg_d = nc.dram_tensor("avgT", [HS, B], F32, kind="ExternalOutput")
    hp_d = nc.dram_tensor("hp_out", [B], F32, kind="ExternalOutput")
    nup_d = nc.dram_tensor("nup_out", [B], F32, kind="ExternalOutput")
    if DEBUG:
        dbg = {k: nc.dram_tensor(k, shp, F32, kind="ExternalOutput") for k, shp in [
            ("dbg_xg", [SL, B]), ("dbg_h1", [HS, B]), ("dbg_y1", [B]),
            ("dbg_h10", [HS, B]), ("dbg_hsum", [HS, B]), ("dbg_rem", [B]),
            ("dbg_fac", [B])]}

    groups = [list(range(NC))]

    with tile.TileContext(nc) as tc:
        with (
            tc.tile_pool(name="wsb", bufs=1) as wsb,
            tc.tile_pool(name="big", bufs=1) as big,
            tc.tile_pool(name="xgp", bufs=1) as xgp,
            tc.tile_pool(name="st", bufs=1) as st,
            tc.tile_pool(name="ps", bufs=1, space="PSUM") as ps,
            tc.tile_pool(name="dram", bufs=1, space="DRAM") as dram,
        ):
            # --- resident tiles ---
            whh = [wsb.tile([128, SL], BF16, tag=f"whh{k}", name=f"whh{k}")
                   for k in range(KT)]
            for k in range(KT):
                nc.sync.dma_start(whh[k][:], whhT_d[128 * k:128 * (k + 1), :])

            xg = [xgp.tile([128, B], F32, tag=f"xg{m}", name=f"xg{m}") for m in range(MT)]

            bias_sb = st.tile([128, MT], F32, name="bias_sb")
            nc.sync.dma_start(bias_sb[:], bias_d.ap().rearrange("(m p) -> p m", p=128))
            whalt_sb = st.tile([128, PT], BF16, name="whalt_sb")
            nc.sync.dma_start(whalt_sb[:], whalt_d.ap().rearrange("(k p) -> p k", p=128))
            bhalt_sb = st.tile([128, 1], F32, name="bhalt_sb")
            nc.sync.dma_start(bhalt_sb[:], bhalt_d[:, :])

            c_t = [st.tile([128, B], F32, tag=f"c{p}", name=f"c{p}") for p in range(PT)]
            hs_t = [st.tile([128, B], F32, tag=f"hs{p}", name=f"hs{p}") for p in range(PT)]
            hn_t = [st.tile([128, B], BF16, tag=f"hn{p}", name=f"hn{p}") for p in range(PT)]
            for p in range(PT):
                nc.vector.memset(c_t[p][:], 0.0)
                nc.vector.memset(hs_t[p][:], 0.0)
            hp_t = st.tile([128, 8], F32, name="hp_t")
            rem_t = st.tile([128, 8], F32, name="rem_t")
            nup_t = st.tile([128, 8], F32, name="nup_t")
            m128 = st.tile([128, 1], F32, name="m128")
            nc.vector.memset(hp_t[:], 0.0)
            nc.vector.memset(rem_t[:], 0.0)
            nc.vector.memset(nup_t[:], 0.0)
            nc.vector.memset(m128[:], 1.0)

            # --- phase 1: xg = W_ih_slice @ x.T + bias (f32r, into resident xg) ---
            with tc.tile_pool(name="xtp", bufs=1) as xtp, \
                 tc.tile_pool(name="wih", bufs=1) as wihp:
                xt = [xtp.tile([128, B], F32R, tag=f"xt{k}", name=f"xt{k}")
                      for k in range(KT)]
                for k in range(KT):
                    nc.sync.dma_start(xt[k][:],
                                      xT_d[128 * k:128 * (k + 1), :].bitcast(F32R))
                for n in range(NH):
                    nsl = slice(n * NW, (n + 1) * NW)
                    pst = [ps.tile([128, NW], F32, tag="ps", bufs=8, name=f"psxg{n}_{m}")
                           for m in range(MT)]
                    for k in range(KT):
                        wt = wihp.tile([128, SL], F32R, tag="wih", bufs=3,
                                       name=f"wih{n}_{k}")
                        nc.sync.dma_start(wt[:],
                                          wihT_d[128 * k:128 * (k + 1), :].bitcast(F32R))
                        for m in range(MT):
                            nc.tensor.matmul(pst[m][:], wt[:, 128 * m:128 * (m + 1)],
                                             xt[k][:, nsl],
                                             start=(k == 0), stop=(k == KT - 1))
                    for m in range(MT):
                        nc.scalar.activation(xg[m][:, nsl], pst[m][:], AF.Identity,
                                             bias=bias_sb[:, m:m + 1])

            # --- phase 2: recurrent steps, pipelined over batch halves ---
            tp_cm = tc.tile_pool(name="tmp", bufs=1)
            tp = tp_cm.__enter__()
            hT = [[None] * KT for _ in range(NH)]
            for t in range(1, STEPS + 1):
                hT_next = [[big.tile([128, NW], BF16, tag=f"h{n}_{k}",
                                     name=f"hT{t}_{n}_{k}")
                            for k in range(KT)] for n in range(NH)] \
                    if t < STEPS else None
                yp_next = st.tile([128, 64], BF16, tag="ypall", bufs=2, name=f"ypall{t}")
                for n in range(NH):
                    nsl = slice(n * NW, (n + 1) * NW)
                    # (A) gate waves for this half
                    for p in range(PT):
                        tmps = []
                        if t == 1:
                            for gi, gname in enumerate("ifgo"):
                                m = 2 * gi + p
                                tt = tp.tile([128, NW], F32, tag=f"t{gi}", bufs=3,
                                             name=f"t{t}_{p}_{n}_{gi}")
                                nc.scalar.activation(
                                    tt[:], xg[m][:, nsl],
                                    AF.Tanh if gname == "g" else AF.Sigmoid)
                                tmps.append(tt)
                        else:
                            pst = [ps.tile([128, NW], F32, tag="ps", bufs=8,
                                           name=f"psg{t}_{p}_{n}_{gi}")
                                   for gi in range(4)]
                            for k in range(KT):
                                for gi in range(4):
                                    m = 2 * gi + p
                                    nc.tensor.matmul(
                                        pst[gi][:],
                                        whh[k][:, 128 * m:128 * (m + 1)],
                                        hT[n][k][:],
                                        start=(k == 0), stop=(k == KT - 1))
                            for gi, gname in enumerate("ifgo"):
                                m = 2 * gi + p
                                tt = tp.tile([128, NW], F32, tag=f"t{gi}", bufs=3,
                                             name=f"t{t}_{p}_{n}_{gi}")
                                nc.vector.tensor_add(tt[:], pst[gi][:], xg[m][:, nsl])
                                nc.scalar.activation(
                                    tt[:], tt[:],
                                    AF.Tanh if gname == "g" else AF.Sigmoid)
                                tmps.append(tt)
                        t_i, t_f, t_g, t_o = tmps
                        nc.vector.tensor_mul(t_i[:], t_i[:], t_g[:])
                        nc.vector.tensor_mul(t_f[:], t_f[:], c_t[p][:, nsl])
                        nc.vector.tensor_add(c_t[p][:, nsl], t_f[:], t_i[:])
                        nc.scalar.activation(t_g[:], c_t[p][:, nsl], AF.Tanh)
                        nc.vector.tensor_mul(hn_t[p][:, nsl], t_o[:], t_g[:])
                        nc.vector.scalar_tensor_tensor(
                            hs_t[p][:, nsl], hn_t[p][:, nsl], m128[:, :],
                            hs_t[p][:, nsl], op0=ALU.mult, op1=ALU.add)

                    # (B) halt partial logits for this half's batch m-tiles
                    yps = ps.tile([128, MH], F32, tag="ps", bufs=8, name=f"yps{t}_{n}")
                    for mi in range(MH):
                        m = n * MH + mi
                        for p in range(PT):
                            nc.tensor.matmul(yps[:, mi:mi + 1],
                                             hn_t[p][:, 128 * m:128 * (m + 1)],
                                             whalt_sb[:, p:p + 1],
                                             start=(p == 0), stop=(p == PT - 1))
                    part_bf = st.tile([128, MH], BF16, tag="part", bufs=2,
                                      name=f"part{t}_{n}")
                    nc.vector.tensor_copy(part_bf[:], yps[:])

                    # (C) exchange: h_new half (t < STEPS) + halt partials
                    if t < STEPS:
                        bounce = dram.tile([BR, NW], BF16, tag="bounce", bufs=2,
                                           name=f"bounce{t}_{n}")
                        for p in range(PT):
                            nc.gpsimd.dma_start(bounce[128 * p:128 * (p + 1), :],
                                                hn_t[p][:, nsl])
                        nc.gpsimd.dma_start(
                            bounce[2 * 128:BR, :].rearrange("o (m q) -> (o q) m", q=128),
                            part_bf[:])
                        agout = dram.tile([NC * BR, NW], BF16, tag="agout", bufs=2,
                                          addr_space="Shared", name=f"agout{t}_{n}")
                        nc.gpsimd.collective_compute(
                            "AllGather", ALU.bypass, replica_groups=groups,
                            ins=[bounce.opt()], outs=[agout.opt()])
                        for k in range(KT):
                            row = (k // PT) * BR + (k % PT) * 128
                            nc.sync.dma_start(hT_next[n][k][:],
                                              agout[row:row + 128, :])
                        for r in range(NC):
                            row = r * BR + 2 * 128
                            nc.sync.dma_start(
                                yp_next[:, 8 * r + MH * n:8 * r + MH * (n + 1)],
                                agout[row:row + 1, :].rearrange(
                                    "o (m q) -> (o q) m", q=128))
                    else:
                        bounce = dram.tile([1, NW], BF16, tag="bouncey", bufs=2,
                                           name=f"bouncey{n}")
                        nc.gpsimd.dma_start(
                            bounce[:, :].rearrange("o (m q) -> (o q) m", q=128),
                            part_bf[:])
                        agout = dram.tile([NC, NW], BF16, tag="agouty", bufs=2,
                                          addr_space="Shared", name=f"agouty{n}")
                        nc.gpsimd.collective_compute(
                            "AllGather", ALU.bypass, replica_groups=groups,
                            ins=[bounce.opt()], outs=[agout.opt()])
                        for r in range(NC):
                            nc.sync.dma_start(
                                yp_next[:, 8 * r + MH * n:8 * r + MH * (n + 1)],
                                agout[r:r + 1, :].rearrange("o (m q) -> (o q) m", q=128))

                # (D) per-step halting-state update (both halves' partials in)
                logit = st.tile([128, 8], F32, tag="logit", name=f"logit{t}")
                nc.vector.tensor_reduce(
                    logit[:], yp_next[:, :].rearrange("p (r j) -> p j r", r=NC),
                    axis=mybir.AxisListType.X, op=ALU.add)
                y_t = st.tile([128, 8], F32, tag="y_t", name=f"y{t}")
                nc.scalar.activation(y_t[:], logit[:], AF.Sigmoid, bias=bhalt_sb[:, :])
                t1 = st.tile([128, 8], F32, tag="t1", name=f"t1_{t}")
                nc.vector.tensor_scalar(t1[:], hp_t[:], -1.0, 1.0, op0=ALU.mult, op1=ALU.add)
                nc.vector.tensor_mul(t1[:], y_t[:], t1[:])          # y*(1-hp)
                nc.vector.scalar_tensor_tensor(hp_t[:], t1[:], m128[:, :], hp_t[:],
                                               op0=ALU.mult, op1=ALU.add)
                nc.vector.tensor_scalar(t1[:], hp_t[:], -1.0, 1.0, op0=ALU.mult, op1=ALU.add)
                nc.vector.scalar_tensor_tensor(rem_t[:], t1[:], m128[:, :], rem_t[:],
                                               op0=ALU.mult, op1=ALU.add)
                nc.vector.tensor_scalar_add(nup_t[:], nup_t[:], m128[:, :])
                # active &= (min(hp) <= THRESH), via -max(-hp)
                hmin = st.tile([128, 1], F32, tag="hmin", name=f"hmin{t}")
                nc.vector.tensor_reduce(hmin[:], hp_t[:], axis=mybir.AxisListType.X,
                                        op=ALU.min)
                nc.vector.tensor_scalar_mul(hmin[:], hmin[:], -1.0)
                nall = st.tile([128, 1], F32, tag="nall", name=f"nall{t}")
                nc.gpsimd.partition_all_reduce(nall[:], hmin[:], channels=128,
                                               reduce_op=bass_isa.ReduceOp.max)
                nc.vector.tensor_scalar(nall[:], nall[:], float(-THRESH), None,
                                        op0=ALU.is_ge)
                nc.vector.tensor_mul(m128[:], m128[:], nall[:])
                hT = hT_next
                if DEBUG and t == 1:
                    for p in range(PT):
                        nc.gpsimd.dma_start(dbg["dbg_h1"][128 * p:128 * (p + 1), :],
                                            hn_t[p][:])
                    nc.sync.dma_start(dbg["dbg_y1"].ap().rearrange("(m q) -> q m", q=128),
                                      y_t[:])
                if DEBUG and t == STEPS:
                    for p in range(PT):
                        nc.gpsimd.dma_start(dbg["dbg_h10"][128 * p:128 * (p + 1), :],
                                            hn_t[p][:])
                        nc.sync.dma_start(dbg["dbg_hsum"][128 * p:128 * (p + 1), :],
                                          hs_t[p][:])
                    nc.sync.dma_start(dbg["dbg_rem"].ap().rearrange("(m q) -> q m", q=128),
                                      rem_t[:])

            # --- phase 3: outputs ---
            fac = st.tile([128, 8], F32, tag="fac", name="fac")
            nc.vector.reciprocal(fac[:], nup_t[:])
            nc.vector.tensor_mul(fac[:], rem_t[:], fac[:])
            fbuf = dram.tile([B], F32, tag="fbuf", name="fbuf")
            nc.sync.dma_start(fbuf[:].rearrange("(m q) -> q m", q=128), fac[:])
            if DEBUG:
                nc.sync.dma_start(dbg["dbg_fac"].ap().rearrange("(m q) -> q m", q=128),
                                  fac[:])
                for m in range(MT):
                    nc.sync.dma_start(dbg["dbg_xg"][128 * m:128 * (m + 1), :], xg[m][:])
            frow = tp.tile([1, B], F32, tag="frow", name="frow")
            nc.sync.dma_start(frow[:], fbuf[:].rearrange("(o b) -> o b", o=1))
            ones_sb = tp.tile([1, 128], F32, tag="ones", name="ones_sb")
            nc.vector.memset(ones_sb[:], 1.0)
            # broadcast factor over partitions via a K=1 matmul, multiply in place
            for n in range(NH):
                nsl = slice(n * NW, (n + 1) * NW)
                fps = ps.tile([128, NW], F32, tag="ps", bufs=8, name=f"fps{n}")
                nc.tensor.matmul(fps[:], ones_sb[:], frow[:, nsl], start=True, stop=True)
                for p in range(PT):
                    av = xgp.tile([128, NW], F32, tag=f"xg{2 * n + p}", name=f"av{n}_{p}")
                    nc.vector.tensor_mul(av[:], hs_t[p][:, nsl], fps[:])
                    nc.sync.dma_start(avg_d[128 * p:128 * (p + 1), nsl], av[:])
            nc.sync.dma_start(hp_d.ap().rearrange("(m q) -> q m", q=128), hp_t[:])
            nc.sync.dma_start(nup_d.ap().rearrange("(m q) -> q m", q=128), nup_t[:])
            tp_cm.__exit__(None, None, None)

    nc.compile()
    return nc


def _shard(x, W_ih, W_hh, b_ih, b_hh, W_halt, b_halt):
    xT = np.ascontiguousarray(x.T)
    bsum = b_ih + b_hh
    in_maps = []
    for r in range(NC):
        rows = np.concatenate([np.arange(g * H + r * HS, g * H + (r + 1) * HS)
                               for g in range(4)])
        in_maps.append({
            "whhT": np.ascontiguousarray(W_hh[rows].T).astype(ml_dtypes.bfloat16),
            "wihT": np.ascontiguousarray(W_ih[rows].T),
            "xT": xT,
            "bias": np.ascontiguousarray(bsum[rows]),
            "whalt": np.ascontiguousarray(
                W_halt[0, r * HS:(r + 1) * HS]).astype(ml_dtypes.bfloat16),
            "bhalt": np.full((128, 1), np.float32(b_halt[0]), np.float32),
        })
    return in_maps


def _run(inputs, trace=False):
    nc = _build()
    in_maps = _shard(**{k: np.asarray(inputs[k], np.float32) for k in
                        ("x", "W_ih", "W_hh", "b_ih", "b_hh", "W_halt", "b_halt")})
    res = run_bass_kernel_spmd(nc, in_maps, core_ids=list(range(NC)), trace=trace)
    avg = np.empty((B, H), np.float32)
    for r in range(NC):
        avg[:, r * HS:(r + 1) * HS] = res.results[r]["avgT"].T
    hp = res.results[0]["hp_out"].reshape(B, 1).astype(np.float32)
    nup = res.results[0]["nup_out"].reshape(B, 1).astype(np.float32)
    return (avg, hp, nup), res


def kernel(**inputs):
    out, _ = _run(inputs, trace=False)
    return out
